# revision 34
# baseline (speedup 1.0000x reference)
"""Trainium2 Bass kernel for nn_ExternalEmbeddingAttention.

Sharding: data-parallel over batch B=8 across 8 NeuronCores (one example per
core); weights replicated.

Host constant-folds (weight-only algebra, computed once in fp64 in kernel()):
  W*   = Wq @ Wk.T        W*T = Wk @ Wq.T        Wvo = Wv @ Wo
  Wcat = [W* | Wvo]   ([H, 2H], shares one stationary per token/k tile)
All large tensors travel and compute in bf16 (fp32 PSUM accumulation); the
per-element error stays ~0.4% which is far inside the 2e-2 scale-relative
gate.  PE cost model: a matmul streams its MOVING free dim at 1 col/cycle, so
the layout is chosen to keep the moving operand wide only where the math
needs it (hs @ Wcat) and N=16 everywhere else:
  ext MLP:  h1T[i,e]  = W1-ktile stationary  x extT moving   (N=16)
            h2T[h,e]  = W2-jtile stationary  x gelu(h1T)     (N=16)
  a_t[h,e]  = WstarT-tile stationary x extLNT moving          (N=16)
  se[s,e]   = hsT-tile stationary x a_t moving                (N=16)
Token phase is split: stage A (per tile: u = hs@W*, ov = hs@Wvo, s_self via a
fused DVE multiply-reduce, ov drained to SBUF) runs as one dense PE stream so
the HAM clock gate stays at 2.4 GHz; stage B (softmax, P@wv', residual+LN)
is interleaved 2-per-A once a_t/wv' exist, so DVE/ACT work hides under the
PE-bound stage-A windows.
"""

import numpy as np
import ml_dtypes

import concourse.bass as bass
import concourse.tile as tile
import concourse.mybir as mybir
from concourse import bacc
from concourse.bass_utils import run_bass_kernel_spmd
from concourse.masks import make_identity
import concourse.bass_utils as _bass_utils

# Walrus's LDWEIGHTS-dedup pass is disabled by default in this harness; with
# fused matmuls every MATMUL re-loads its stationary operand otherwise.
_orig_run_command = _bass_utils.run_command


_LDW_OPT = False  # walrus ldw-opt rejects bf16 (FWL-format) LDWEIGHTS


def _run_command_ldwopt(argv, **kwargs):
    if _LDW_OPT:
        argv = ["--enable-ldw-opt=true" if a == "--enable-ldw-opt=false"
                else a for a in argv]
    return _orig_run_command(argv, **kwargs)


if _bass_utils.run_command is not _run_command_ldwopt:
    _bass_utils.run_command = _run_command_ldwopt

# Steer the act-table chooser: Exp and Ln both live in
# natural_log_exp_and_others; restrict them to that shared set so the
# scheduler never reloads activation tables mid-loop.
from concourse.hw_specs import get_activation_tables as _gat


def _steer_act_tables(arch="gen3"):
    t = _gat(arch)   # functools.cache -> in-place mutation persists
    for name, funcs in t.items():
        if name != "natural_log_exp_and_others":
            funcs.discard(mybir.ActivationFunctionType.Exp)
            funcs.discard(mybir.ActivationFunctionType.Ln)


_steer_act_tables()

F32 = mybir.dt.float32
BF = mybir.dt.bfloat16
F8 = mybir.dt.float8e4
W8SCALE = 16.0
AF = mybir.ActivationFunctionType
OP = mybir.AluOpType

B, S, H, E, I = 8, 2048, 768, 16, 3072
EPS = 1e-12
P = 128
KO = H // P          # 6 k-tiles over a 768 dim
TT = S // P          # 16 token tiles
JO = I // P          # 24 i-tiles over 3072
ISL = I // B         # 384-wide I-shard per core (collective ext MLP)
JS = ISL // P        # 3 i-tiles per shard
H2 = 2 * H
NPBF = np.dtype(ml_dtypes.bfloat16)


_STAGE = "full"   # debug bisect knob: dma | A | mlp | se | full


def _build(use_bias: dict):
    nc = bacc.Bacc(num_devices=8)

    hs_d = nc.dram_tensor("hs", [S, H], BF, kind="ExternalInput")
    ext_d = nc.dram_tensor("ext", [E, H], BF, kind="ExternalInput")
    dl_d = nc.dram_tensor("dl", [E, 1], F32, kind="ExternalInput")
    wcat_d = nc.dram_tensor("Wcat", [H, H2], BF, kind="ExternalInput")
    wstarT_d = nc.dram_tensor("WstarT", [H, H], BF, kind="ExternalInput")
    w1_d = nc.dram_tensor("W1", [H, I], F8, kind="ExternalInput")
    w2_d = nc.dram_tensor("W2", [I, H], F8, kind="ExternalInput")
    bias_d = {}
    for nm, sz in (("b1", I), ("b2", H), ("bo", H), ("mlp_g", H),
                   ("mlp_b", H), ("ln_g", H), ("ln_b", H), ("dvec", H),
                   ("c0", 1), ("wkbq", H), ("bqbk", 1), ("bvwo", H)):
        if use_bias.get(nm):
            bias_d[nm] = nc.dram_tensor(nm, [1, sz], F32, kind="ExternalInput")
    out_d = nc.dram_tensor("out", [S, H], BF, kind="ExternalOutput")

    hs_r = hs_d.rearrange("(tt p) h -> p tt h", p=128)
    w1_r = w1_d.rearrange("(ko p) n -> p ko n", p=128)
    w2_r = w2_d.rearrange("(jo p) n -> p jo n", p=128)
    out_r = out_d.rearrange("(tt p) h -> p tt h", p=128)

    with tile.TileContext(nc) as tc:
        with tc.tile_pool(name="persist", bufs=1) as persist, \
             tc.tile_pool(name="work", bufs=2) as work, \
             tc.tile_pool(name="wpool", bufs=1) as wpool, \
             tc.tile_pool(name="uo_ps", bufs=2, space="PSUM") as uo_pool:

            # ---------------- persistent tiles ----------------
            ident_f = persist.tile([128, 128], F32, tag="ident_f")
            make_identity(nc, ident_f)
            ident = persist.tile([128, 128], BF, tag="ident")
            nc.vector.tensor_copy(ident, ident_f)
            eps_t = persist.tile([128, 1], F32, tag="eps")
            nc.vector.memset(eps_t, EPS)
            dl_t = persist.tile([E, 1], F32, tag="dl")
            nc.sync.dma_start(dl_t, dl_d[:])

            bias_t = {}
            for nm, d in bias_d.items():
                sz = d.shape[1]
                pp = E if nm in ("b1", "b2", "mlp_g", "mlp_b", "wkbq") else P
                t = persist.tile([pp, sz], F32, tag=f"bias_{nm}",
                                 name=f"bias_{nm}")
                nc.gpsimd.dma_start(t, d[:].to_broadcast((pp, sz)))
                bias_t[nm] = t

            hs_sb = persist.tile([128, TT, H], BF, tag="hs")
            hsT = persist.tile([128, KO, S], BF, tag="hsT")
            wcat_sb = persist.tile([128, KO, H2], BF, tag="wcat")
            wstarT_sb = persist.tile([128, KO, H], BF, tag="wstarT")
            ov_sb = persist.tile([128, TT, H], BF, tag="ov_sb")
            ss_all = persist.tile([128, TT], F32, tag="ss_all")
            se_sb = persist.tile([128, TT, E], F32, tag="se_sb")
            # E=16-row operands are zero-padded to 128 partitions so every
            # LDWEIGHTS is K=128 (K<128 LDW gets row-tiled, which walrus's
            # ldw-opt pass rejects); the zero rows contribute nothing
            ext_t = persist.tile([128, H], BF, tag="ext_t")
            nc.vector.memset(ext_t, 0.0)
            ext_all_sb = persist.tile([128, H], BF, tag="ext_all")
            extT = persist.tile([128, KO, 128], BF, tag="extT")
            extLN = persist.tile([128, H], BF, tag="extLN")
            nc.vector.memset(extLN, 0.0)
            extLNT = persist.tile([128, KO, E], BF, tag="extLNT")
            a_t = persist.tile([128, KO, E], BF, tag="a_t")
            wvext = persist.tile([128, H], BF, tag="wvext")
            nc.vector.memset(wvext, 0.0)
            pgT_pad = persist.tile([128, 128], BF, tag="pgT_pad")
            nc.vector.memset(pgT_pad, 0.0)
            pg_ring = persist.tile([128, 4, E], BF, tag="pg_ring")
            p0_ring = persist.tile([128, 4], F32, tag="p0_ring")
            cvec_bc = (persist.tile([128, E], F32, tag="cvec_bc")
                       if use_bias.get("wkbq") else None)

            # ---------------- DMA issue (sync ring, priority order) -------
            nc.sync.dma_start(ext_t[:E], ext_d[:])
            for tt in range(2):
                nc.sync.dma_start(hs_sb[:, tt], hs_r[:, tt])
            nc.sync.dma_start(wcat_sb,
                              wcat_d.rearrange("(ko p) n -> p ko n", p=128))
            w1_sb = wpool.tile([128, KO, I], F8, tag="w1")
            w2_sb = wpool.tile([128, JO, H], F8, tag="w2")
            for c in range(2):
                nc.sync.dma_start(w1_sb[:, :, c * 1536:(c + 1) * 1536],
                                  w1_r[:, :, c * 1536:(c + 1) * 1536])
            nc.sync.dma_start(wstarT_sb,
                              wstarT_d.rearrange("(ko p) n -> p ko n", p=128))
            for c in range(2):
                nc.sync.dma_start(w2_sb[:, c * 12:(c + 1) * 12],
                                  w2_r[:, c * 12:(c + 1) * 12])
            for tt in range(2, TT):
                nc.sync.dma_start(hs_sb[:, tt], hs_r[:, tt])

            # ---------------- emission helpers ----------------
            def tp128(pool, dst_ap, src_ap, eng):
                """Transpose [p,f]->[f,p] as a plain matmul against the
                identity (identical math to nc.tensor.transpose, but the
                ordinary LDWEIGHTS form keeps walrus's ldw-opt pass happy)."""
                pdim = src_ap.shape[-1]
                fdim = src_ap.shape[0]
                ps = pool.tile([128, 128], F32, tag="tp")
                nc.tensor.matmul(ps[:pdim, :fdim], src_ap,
                                 ident[:fdim, :fdim], start=True, stop=True)
                if eng == "act":
                    nc.scalar.copy(dst_ap, ps[:pdim, :fdim])
                else:
                    nc.vector.tensor_copy(dst_ap, ps[:pdim, :fdim])

            def tpPad(pool, dst_ap, src_ap, rows, eng):
                """Transpose a zero-padded [128,128] block, keeping only the
                first `rows` output columns (moving N=rows)."""
                ps = pool.tile([128, 128], F32, tag="tp")
                nc.tensor.matmul(ps[:128, :rows], src_ap,
                                 ident[:128, :rows], start=True, stop=True)
                if eng == "act":
                    nc.scalar.copy(dst_ap, ps[:128, :rows])
                else:
                    nc.vector.tensor_copy(dst_ap, ps[:128, :rows])

            def emit_hsT(tt, pool):
                for k in range(KO):
                    tp128(pool, hsT[:, k, tt * P:(tt + 1) * P],
                          hs_sb[:, tt, k * P:(k + 1) * P],
                          "act" if (tt * KO + k) % 2 else "dve")

            def emit_A(tt):
                """u/ov matmuls + s_self + ov drain for one token tile."""
                uo = uo_pool.tile([128, H2], F32, tag="uo")
                for k in range(KO):
                    lhs = hsT[:, k, tt * P:(tt + 1) * P]
                    for c in range(3):
                        nc.tensor.matmul(
                            uo[:, c * 512:(c + 1) * 512], lhs,
                            wcat_sb[:, k, c * 512:(c + 1) * 512],
                            start=(k == 0), stop=(k == KO - 1))
                u_ps = uo[:, 0:H]
                ov_ps = uo[:, H:H2]
                if use_bias.get("dvec"):
                    nc.vector.tensor_add(u_ps, u_ps, bias_t["dvec"])
                scr = work.tile([128, H], BF, tag="scr")
                nc.vector.tensor_mul(scr, u_ps, hs_sb[:, tt])
                nc.vector.reduce_sum(ss_all[:, tt:tt + 1], scr,
                                     axis=mybir.AxisListType.X)
                nc.scalar.copy(ov_sb[:, tt], ov_ps)

            def emit_mlp_w1(mp):
                h1_ps = mp.tile([128, JS, 128], F32, tag="h1T")
                for j in range(JS):
                    for k in range(KO):
                        nc.tensor.matmul(
                            h1_ps[:, j], w1_sb[:, k, j * P:(j + 1) * P],
                            extT[:, k], start=(k == 0), stop=(k == KO - 1))
                h1g = wpool.tile([128, JS, 128], BF, tag="h1g")
                if use_bias.get("b1"):
                    b1v = wpool.tile([128, JS, 1], F32, tag="b1v")
                    nc.sync.dma_start(
                        b1v, bias_d["b1"][:].rearrange(
                            "o (jo p) -> p jo o", p=128))
                    for j in range(JS):
                        nc.scalar.activation(h1g[:, j], h1_ps[:, j], AF.Gelu,
                                             bias=b1v[:, j])
                else:
                    nc.scalar.activation(h1g, h1_ps, AF.Gelu)
                return h1g

            def emit_mlp_w2(h1g, mp):
                # partial h2 for ALL examples using this core's I-shard,
                # then an 8-core ReduceScatter hands each core its own
                # example's 16 rows
                h2_ps = mp.tile([128, H], F32, tag="h2")
                for j in range(JS):
                    for off, ln in ((0, 512), (512, 256)):
                        nc.tensor.matmul(
                            h2_ps[:, off:off + ln], h1g[:, j],
                            w2_sb[:, j, off:off + ln],
                            start=(j == 0), stop=(j == JS - 1))
                h2sb = wpool.tile([128, H], F32, tag="h2sb")
                nc.scalar.copy(h2sb, h2_ps)
                nc.sync.dma_start(h2p_d[:], h2sb)
                nc.gpsimd.collective_compute(
                    "ReduceScatter", OP.add,
                    [[0, 1, 2, 3, 4, 5, 6, 7]],
                    ins=[h2p_d[:]], outs=[h2s_d[:]], cc_dim="Partition")
                z0 = wpool.tile([E, H], F32, tag="z0")
                nc.sync.dma_start(z0, h2s_d[:])
                # residual + LN over free dim (16 partitions)
                z = wpool.tile([E, H], F32, tag="z")
                nc.vector.tensor_add(z, z0, ext_t[:E])
                if use_bias.get("b2"):
                    nc.vector.tensor_add(z, z, bias_t["b2"][:E])
                stats = wpool.tile([E, 3, 6], F32, tag="st")
                for g in range(3):
                    nc.vector.bn_stats(stats[:, g],
                                       z[:, g * 256:(g + 1) * 256])
                mv = wpool.tile([E, 2], F32, tag="mv")
                nc.vector.bn_aggr(mv, stats)
                lnv = wpool.tile([E, 1], F32, tag="lnv")
                nc.scalar.activation(lnv, mv[:, 1:2], AF.Ln, bias=eps_t[:E])
                rs = wpool.tile([E, 1], F32, tag="rs")
                nc.scalar.activation(rs, lnv, AF.Exp, scale=-0.5)
                nc.vector.tensor_scalar(extLN[:E], z, mv[:, 0:1], rs,
                                        op0=OP.subtract, op1=OP.mult)
                if use_bias.get("mlp_g"):
                    nc.vector.tensor_mul(extLN[:E], extLN[:E],
                                         bias_t["mlp_g"][:E])
                if use_bias.get("mlp_b"):
                    nc.vector.tensor_add(extLN[:E], extLN[:E],
                                         bias_t["mlp_b"][:E])

            def emit_p2(mp):
                for k in range(KO):
                    tpPad(mp, extLNT[:, k], extLN[:, k * P:(k + 1) * P],
                          E, "act" if k % 2 else "dve")
                # a_t[:, k][h, e] = sum_h' W*[kh, h'] extLN[e, h']
                at_ps = mp.tile([128, KO, E], F32, tag="at")
                for k in range(KO):
                    for kp in range(KO):
                        nc.tensor.matmul(
                            at_ps[:, k],
                            wstarT_sb[:, kp, k * P:(k + 1) * P],
                            extLNT[:, kp], start=(kp == 0),
                            stop=(kp == KO - 1))
                nc.vector.tensor_copy(a_t, at_ps)
                # cvec[e] = bq . k_ext[e]  (general-bias path)
                if use_bias.get("wkbq"):
                    scrq = wpool.tile([E, H], F32, tag="cscr")
                    cv = wpool.tile([E, 1], F32, tag="cv")
                    nc.vector.tensor_mul(scrq, extLN[:E], bias_t["wkbq"][:E])
                    nc.vector.reduce_sum(cv, scrq, axis=mybir.AxisListType.X)
                    nc.vector.tensor_scalar_add(cv, cv, bias_t["bqbk"][:E])
                    cvp = mp.tile([128, 128], F32, tag="cvp")
                    nc.tensor.transpose(cvp[:1, :E], cv, ident_f[:E, :E])
                    cvr = wpool.tile([1, E], F32, tag="cvr")
                    nc.vector.tensor_copy(cvr, cvp[:1, :E])
                    nc.gpsimd.dma_start(cvec_bc, cvr.to_broadcast((128, E)))

            def emit_wv(mp):
                # wv' = gamma * (extLN @ Wvo) reusing the Wvo half of wcat
                wv_ps = mp.tile([E, 384], F32, tag="wv")
                for hf in range(2):
                    for k in range(KO):
                        nc.tensor.matmul(
                            wv_ps, extLNT[:, k],
                            wcat_sb[:, k, H + hf * 384:H + (hf + 1) * 384],
                            start=(k == 0), stop=(k == KO - 1))
                    if use_bias.get("bvwo"):
                        nc.vector.tensor_add(
                            wv_ps, wv_ps,
                            bias_t["bvwo"][:E, hf * 384:(hf + 1) * 384])
                    nc.vector.tensor_scalar_mul(
                        wvext[:E, hf * 384:(hf + 1) * 384], wv_ps, dl_t)

            def emit_se_all(mp):
                se_ps = mp.tile([128, E], F32, tag="se")
                for tt in range(TT):
                    for k in range(KO):
                        nc.tensor.matmul(se_ps,
                                         hsT[:, k, tt * P:(tt + 1) * P],
                                         a_t[:, k], start=(k == 0),
                                         stop=(k == KO - 1))
                    if use_bias.get("wkbq"):
                        nc.vector.tensor_add(se_sb[:, tt], se_ps, cvec_bc)
                    else:
                        nc.vector.tensor_copy(se_sb[:, tt], se_ps)

            def emit_Bs(tt):
                """softmax scalars for one token tile -> pg/p0 rings."""
                eext = work.tile([128, E], F32, tag="eext")
                zext = work.tile([128, 1], F32, tag="zext")
                nc.scalar.activation(eext, se_sb[:, tt], AF.Exp,
                                     accum_out=zext)
                e0 = work.tile([128, 1], F32, tag="e0")
                if use_bias.get("c0"):
                    nc.scalar.activation(e0, ss_all[:, tt:tt + 1], AF.Exp,
                                         bias=bias_t["c0"])
                else:
                    nc.scalar.activation(e0, ss_all[:, tt:tt + 1], AF.Exp)
                z_t = work.tile([128, 1], F32, tag="z")
                nc.vector.tensor_add(z_t, zext, e0)
                rz = work.tile([128, 1], F32, tag="rz")
                nc.vector.reciprocal(rz, z_t)
                nc.vector.tensor_mul(p0_ring[:, tt % 4:tt % 4 + 1], e0, rz)
                nc.vector.tensor_scalar_mul(pg_ring[:, tt % 4], eext, rz)

            def emit_Bt(tt, pgt_pool):
                """P@wv' + output dense tail for one token tile."""
                pgT_ps = pgt_pool.tile([E, 128], F32, tag="pgT")
                nc.tensor.matmul(pgT_ps, pg_ring[:, tt % 4], ident,
                                 start=True, stop=True)
                nc.scalar.copy(pgT_pad[:E], pgT_ps)
                # sb1 = p0 * ov   (ACT Copy+scale, from SBUF)
                sb1 = work.tile([128, H], BF, tag="sb1")
                nc.scalar.activation(sb1, ov_sb[:, tt], AF.Copy,
                                     scale=p0_ring[:, tt % 4:tt % 4 + 1])
                # Pv overwrites banks 1-2 of a retired uo buffer: address
                # overlap with both the u tail and the ov head orders it
                # after every reader of that buffer
                pv = uo_pool.tile([128, H2], F32, tag="uo")
                for off, ln in ((512, 512), (1024, 256)):
                    nc.tensor.matmul(pv[:, off:off + ln], pgT_pad,
                                     wvext[:, off - 512:off - 512 + ln],
                                     start=True, stop=True)
                out2 = work.tile([128, H], BF, tag="out2")
                nc.vector.tensor_add(out2, sb1, pv[:, 512:512 + H])
                if use_bias.get("bo"):
                    nc.vector.tensor_add(out2, out2, bias_t["bo"])
                sbz = work.tile([128, H], BF, tag="sbz")
                nc.gpsimd.tensor_add(sbz, out2, hs_sb[:, tt])
                # LayerNorm over H; rstd = Exp(-0.5 * Ln(var + eps))
                stats = work.tile([128, 2, 6], F32, tag="lnst")
                for g in range(2):
                    nc.vector.bn_stats(stats[:, g],
                                       sbz[:, g * 384:(g + 1) * 384])
                mv = work.tile([128, 2], F32, tag="lnmv")
                nc.vector.bn_aggr(mv, stats)
                lnv = work.tile([128, 1], F32, tag="lnv")
                nc.scalar.activation(lnv, mv[:, 1:2], AF.Ln, bias=eps_t)
                rs = work.tile([128, 1], F32, tag="lnrs")
                nc.scalar.activation(rs, lnv, AF.Exp, scale=-0.5)
                fin = work.tile([128, H], BF, tag="fin")
                nc.vector.tensor_scalar(fin, sbz, mv[:, 0:1], rs,
                                        op0=OP.subtract, op1=OP.mult)
                if use_bias.get("ln_g"):
                    nc.vector.tensor_mul(fin, fin, bias_t["ln_g"])
                if use_bias.get("ln_b"):
                    nc.vector.tensor_add(fin, fin, bias_t["ln_b"])
                nc.scalar.dma_start(out_r[:, tt], fin)

            # ---------------- schedule ----------------
            stg = {"dma": 0, "A": 1, "mlp": 2, "se": 3, "full": 4}[_STAGE]
            with tc.tile_pool(name="tpX", bufs=1, space="PSUM") as tpx:
                for k in range(KO):
                    tp128(tpx, extT[:, k], ext_all_sb[:, k * P:(k + 1) * P],
                          "act" if k % 2 else "dve")
            if stg >= 2:
                with tc.tile_pool(name="mlp1", bufs=1, space="PSUM") as mp:
                    h1g = emit_mlp_w1(mp)
                with tc.tile_pool(name="mlp2", bufs=1, space="PSUM") as mp:
                    emit_mlp_w2(h1g, mp)
            with tc.tile_pool(name="tpA", bufs=2, space="PSUM") as tpa:
                emit_hsT(0, tpa)
                emit_hsT(1, tpa)
                if stg >= 1:
                    emit_A(0)
                emit_hsT(2, tpa)
                emit_hsT(3, tpa)
                if stg >= 1:
                    emit_A(1)
                emit_hsT(4, tpa)
                emit_hsT(5, tpa)
                if stg >= 1:
                    emit_A(2)
                emit_hsT(6, tpa)
                emit_hsT(7, tpa)
                if stg >= 1:
                    emit_A(3)
                    emit_A(4)
            if stg >= 1:
                emit_A(5)
                emit_A(6)
            with tc.tile_pool(name="tpB", bufs=2, space="PSUM") as tpb:
                for tt in range(8, TT):
                    emit_hsT(tt, tpb)
            if stg >= 2:
                with tc.tile_pool(name="p2a", bufs=1, space="PSUM") as mp:
                    emit_p2(mp)
                with tc.tile_pool(name="p2b", bufs=1, space="PSUM") as mp:
                    emit_wv(mp)
            if stg >= 3:
                with tc.tile_pool(name="sep", bufs=1, space="PSUM") as mp:
                    emit_se_all(mp)
            if stg >= 4:
                with tc.tile_pool(name="pgt_ps", bufs=1,
                                  space="PSUM") as pgt_pool:
                    sq = list(range(TT))   # softmax pending
                    tq = []                # tail pending
                    for tt in range(7, TT):
                        emit_A(tt)
                        for _ in range(2):
                            if tq:
                                emit_Bt(tq.pop(0), pgt_pool)
                        for _ in range(2):
                            if sq and sq[0] <= tt:
                                j = sq.pop(0)
                                emit_Bs(j)
                                tq.append(j)
                    while sq or tq:
                        if tq:
                            emit_Bt(tq.pop(0), pgt_pool)
                        if sq:
                            j = sq.pop(0)
                            emit_Bs(j)
                            tq.append(j)
            else:
                if stg >= 1:
                    for tt in range(7, TT):
                        emit_A(tt)
                for tt in range(TT):
                    src = ov_sb[:, tt] if stg >= 1 else hs_sb[:, tt]
                    nc.scalar.dma_start(out_r[:, tt], src)


# revision 35
# speedup vs baseline: 1.3504x; 1.3504x over previous
"""Trainium2 Bass kernel for nn_ExternalEmbeddingAttention.

Sharding: data-parallel over batch B=8 across 8 NeuronCores (one example per
core); weights replicated.

Host constant-folds (weight-only algebra, computed once in fp64 in kernel()):
  W*   = Wq @ Wk.T        W*T = Wk @ Wq.T        Wvo = Wv @ Wo
  Wcat = [W* | Wvo]   ([H, 2H], shares one stationary per token/k tile)
All large tensors travel and compute in bf16 (fp32 PSUM accumulation); the
per-element error stays ~0.4% which is far inside the 2e-2 scale-relative
gate.  PE cost model: a matmul streams its MOVING free dim at 1 col/cycle, so
the layout is chosen to keep the moving operand wide only where the math
needs it (hs @ Wcat) and N=16 everywhere else:
  ext MLP:  h1T[i,e]  = W1-ktile stationary  x extT moving   (N=16)
            h2T[h,e]  = W2-jtile stationary  x gelu(h1T)     (N=16)
  a_t[h,e]  = WstarT-tile stationary x extLNT moving          (N=16)
  se[s,e]   = hsT-tile stationary x a_t moving                (N=16)
Token phase is split: stage A (per tile: u = hs@W*, ov = hs@Wvo, s_self via a
fused DVE multiply-reduce, ov drained to SBUF) runs as one dense PE stream so
the HAM clock gate stays at 2.4 GHz; stage B (softmax, P@wv', residual+LN)
is interleaved 2-per-A once a_t/wv' exist, so DVE/ACT work hides under the
PE-bound stage-A windows.
"""

import numpy as np
import ml_dtypes

import concourse.bass as bass
import concourse.tile as tile
import concourse.mybir as mybir
from concourse import bacc
from concourse.bass_utils import run_bass_kernel_spmd
from concourse.masks import make_identity
import concourse.bass_utils as _bass_utils

# Walrus's LDWEIGHTS-dedup pass is disabled by default in this harness; with
# fused matmuls every MATMUL re-loads its stationary operand otherwise.
_orig_run_command = _bass_utils.run_command


_LDW_OPT = False  # walrus ldw-opt rejects bf16 (FWL-format) LDWEIGHTS


def _run_command_ldwopt(argv, **kwargs):
    if _LDW_OPT:
        argv = ["--enable-ldw-opt=true" if a == "--enable-ldw-opt=false"
                else a for a in argv]
    return _orig_run_command(argv, **kwargs)


if _bass_utils.run_command is not _run_command_ldwopt:
    _bass_utils.run_command = _run_command_ldwopt

# Steer the act-table chooser: Exp and Ln both live in
# natural_log_exp_and_others; restrict them to that shared set so the
# scheduler never reloads activation tables mid-loop.
from concourse.hw_specs import get_activation_tables as _gat


def _steer_act_tables(arch="gen3"):
    t = _gat(arch)   # functools.cache -> in-place mutation persists
    for name, funcs in t.items():
        if name != "natural_log_exp_and_others":
            funcs.discard(mybir.ActivationFunctionType.Exp)
            funcs.discard(mybir.ActivationFunctionType.Ln)


_steer_act_tables()

F32 = mybir.dt.float32
BF = mybir.dt.bfloat16
F8 = mybir.dt.float8e4
W8SCALE = 16.0
AF = mybir.ActivationFunctionType
OP = mybir.AluOpType

B, S, H, E, I = 8, 2048, 768, 16, 3072
EPS = 1e-12
P = 128
KO = H // P          # 6 k-tiles over a 768 dim
TT = S // P          # 16 token tiles
JO = I // P          # 24 i-tiles over 3072
ISL = I // B         # 384-wide I-shard per core (collective ext MLP)
JS = ISL // P        # 3 i-tiles per shard
H2 = 2 * H
NPBF = np.dtype(ml_dtypes.bfloat16)


_STAGE = "full"   # debug bisect knob: dma | A | mlp | se | full


def _build(use_bias: dict):
    nc = bacc.Bacc(num_devices=8)

    hs_d = nc.dram_tensor("hs", [S, H], BF, kind="ExternalInput")
    ext_d = nc.dram_tensor("ext", [E, H], BF, kind="ExternalInput")
    dl_d = nc.dram_tensor("dl", [E, 1], F32, kind="ExternalInput")
    wcat_d = nc.dram_tensor("Wcat", [H, H2], BF, kind="ExternalInput")
    wstarT_d = nc.dram_tensor("WstarT", [H, H], BF, kind="ExternalInput")
    w1_d = nc.dram_tensor("W1", [H, I], F8, kind="ExternalInput")
    w2_d = nc.dram_tensor("W2", [I, H], F8, kind="ExternalInput")
    bias_d = {}
    for nm, sz in (("b1", I), ("b2", H), ("bo", H), ("mlp_g", H),
                   ("mlp_b", H), ("ln_g", H), ("ln_b", H), ("dvec", H),
                   ("c0", 1), ("wkbq", H), ("bqbk", 1), ("bvwo", H)):
        if use_bias.get(nm):
            bias_d[nm] = nc.dram_tensor(nm, [1, sz], F32, kind="ExternalInput")
    out_d = nc.dram_tensor("out", [S, H], BF, kind="ExternalOutput")

    hs_r = hs_d.rearrange("(tt p) h -> p tt h", p=128)
    w1_r = w1_d.rearrange("(ko p) n -> p ko n", p=128)
    w2_r = w2_d.rearrange("(jo p) n -> p jo n", p=128)
    out_r = out_d.rearrange("(tt p) h -> p tt h", p=128)

    with tile.TileContext(nc) as tc:
        with tc.tile_pool(name="persist", bufs=1) as persist, \
             tc.tile_pool(name="work", bufs=2) as work, \
             tc.tile_pool(name="wpool", bufs=1) as wpool, \
             tc.tile_pool(name="uo_ps", bufs=2, space="PSUM") as uo_pool:

            # ---------------- persistent tiles ----------------
            ident_f = persist.tile([128, 128], F32, tag="ident_f")
            make_identity(nc, ident_f)
            ident = persist.tile([128, 128], BF, tag="ident")
            nc.vector.tensor_copy(ident, ident_f)
            eps_t = persist.tile([128, 1], F32, tag="eps")
            nc.vector.memset(eps_t, EPS)
            dl_t = persist.tile([E, 1], F32, tag="dl")
            nc.sync.dma_start(dl_t, dl_d[:])

            bias_t = {}
            for nm, d in bias_d.items():
                sz = d.shape[1]
                pp = E if nm in ("b1", "b2", "mlp_g", "mlp_b", "wkbq") else P
                t = persist.tile([pp, sz], F32, tag=f"bias_{nm}",
                                 name=f"bias_{nm}")
                nc.gpsimd.dma_start(t, d[:].to_broadcast((pp, sz)))
                bias_t[nm] = t

            hs_sb = persist.tile([128, TT, H], BF, tag="hs")
            hsT = persist.tile([128, KO, S], BF, tag="hsT")
            wcat_sb = persist.tile([128, KO, H2], BF, tag="wcat")
            wstarT_sb = persist.tile([128, KO, H], BF, tag="wstarT")
            ov_sb = persist.tile([128, TT, H], BF, tag="ov_sb")
            ss_all = persist.tile([128, TT], F32, tag="ss_all")
            se_sb = persist.tile([128, TT, E], F32, tag="se_sb")
            # E=16-row operands are zero-padded to 128 partitions so every
            # LDWEIGHTS is K=128 (K<128 LDW gets row-tiled, which walrus's
            # ldw-opt pass rejects); the zero rows contribute nothing
            ext_t = persist.tile([128, H], BF, tag="ext_t")
            nc.vector.memset(ext_t, 0.0)
            ext_all_sb = persist.tile([128, H], BF, tag="ext_all")
            extT = persist.tile([128, KO, 128], BF, tag="extT")
            extLN = persist.tile([128, H], BF, tag="extLN")
            nc.vector.memset(extLN, 0.0)
            extLNT = persist.tile([128, KO, E], BF, tag="extLNT")
            a_t = persist.tile([128, KO, E], BF, tag="a_t")
            wvext = persist.tile([128, H], BF, tag="wvext")
            nc.vector.memset(wvext, 0.0)
            pgT_pad = persist.tile([128, 128], BF, tag="pgT_pad")
            nc.vector.memset(pgT_pad, 0.0)
            pg_ring = persist.tile([128, 4, E], BF, tag="pg_ring")
            p0_ring = persist.tile([128, 4], F32, tag="p0_ring")
            cvec_bc = (persist.tile([128, E], F32, tag="cvec_bc")
                       if use_bias.get("wkbq") else None)

            # ---------------- DMA issue (sync ring, priority order) -------
            nc.sync.dma_start(ext_t[:E], ext_d[:])
            for tt in range(2):
                nc.sync.dma_start(hs_sb[:, tt], hs_r[:, tt])
            nc.sync.dma_start(wcat_sb,
                              wcat_d.rearrange("(ko p) n -> p ko n", p=128))
            w1_sb = wpool.tile([128, KO, I], F8, tag="w1")
            w2_sb = wpool.tile([128, JO, H], F8, tag="w2")
            for c in range(2):
                nc.sync.dma_start(w1_sb[:, :, c * 1536:(c + 1) * 1536],
                                  w1_r[:, :, c * 1536:(c + 1) * 1536])
            nc.sync.dma_start(wstarT_sb,
                              wstarT_d.rearrange("(ko p) n -> p ko n", p=128))
            for c in range(2):
                nc.sync.dma_start(w2_sb[:, c * 12:(c + 1) * 12],
                                  w2_r[:, c * 12:(c + 1) * 12])
            for tt in range(2, TT):
                nc.sync.dma_start(hs_sb[:, tt], hs_r[:, tt])

            # ---------------- emission helpers ----------------
            def tp128(pool, dst_ap, src_ap, eng):
                """Transpose [p,f]->[f,p] as a plain matmul against the
                identity (identical math to nc.tensor.transpose, but the
                ordinary LDWEIGHTS form keeps walrus's ldw-opt pass happy)."""
                pdim = src_ap.shape[-1]
                fdim = src_ap.shape[0]
                ps = pool.tile([128, 128], F32, tag="tp")
                nc.tensor.matmul(ps[:pdim, :fdim], src_ap,
                                 ident[:fdim, :fdim], start=True, stop=True)
                if eng == "act":
                    nc.scalar.copy(dst_ap, ps[:pdim, :fdim])
                else:
                    nc.vector.tensor_copy(dst_ap, ps[:pdim, :fdim])

            def tpPad(pool, dst_ap, src_ap, rows, eng):
                """Transpose a zero-padded [128,128] block, keeping only the
                first `rows` output columns (moving N=rows)."""
                ps = pool.tile([128, 128], F32, tag="tp")
                nc.tensor.matmul(ps[:128, :rows], src_ap,
                                 ident[:128, :rows], start=True, stop=True)
                if eng == "act":
                    nc.scalar.copy(dst_ap, ps[:128, :rows])
                else:
                    nc.vector.tensor_copy(dst_ap, ps[:128, :rows])

            def emit_hsT(tt, pool):
                for k in range(KO):
                    tp128(pool, hsT[:, k, tt * P:(tt + 1) * P],
                          hs_sb[:, tt, k * P:(k + 1) * P],
                          "act" if (tt * KO + k) % 2 else "dve")

            def emit_A(tt):
                """u/ov matmuls + s_self + ov drain for one token tile."""
                uo = uo_pool.tile([128, H2], F32, tag="uo")
                for k in range(KO):
                    lhs = hsT[:, k, tt * P:(tt + 1) * P]
                    for c in range(3):
                        nc.tensor.matmul(
                            uo[:, c * 512:(c + 1) * 512], lhs,
                            wcat_sb[:, k, c * 512:(c + 1) * 512],
                            start=(k == 0), stop=(k == KO - 1))
                u_ps = uo[:, 0:H]
                ov_ps = uo[:, H:H2]
                if use_bias.get("dvec"):
                    nc.vector.tensor_add(u_ps, u_ps, bias_t["dvec"])
                scr = work.tile([128, H], BF, tag="scr")
                nc.vector.tensor_mul(scr, u_ps, hs_sb[:, tt])
                nc.vector.reduce_sum(ss_all[:, tt:tt + 1], scr,
                                     axis=mybir.AxisListType.X)
                nc.scalar.copy(ov_sb[:, tt], ov_ps)

            def emit_mlp_w1(mp):
                h1_ps = mp.tile([128, JS, 128], F32, tag="h1T")
                for j in range(JS):
                    for k in range(KO):
                        nc.tensor.matmul(
                            h1_ps[:, j], w1_sb[:, k, j * P:(j + 1) * P],
                            extT[:, k], start=(k == 0), stop=(k == KO - 1))
                h1g = wpool.tile([128, JS, 128], BF, tag="h1g")
                if use_bias.get("b1"):
                    b1v = wpool.tile([128, JS, 1], F32, tag="b1v")
                    nc.sync.dma_start(
                        b1v, bias_d["b1"][:].rearrange(
                            "o (jo p) -> p jo o", p=128))
                    for j in range(JS):
                        nc.scalar.activation(h1g[:, j], h1_ps[:, j], AF.Gelu,
                                             bias=b1v[:, j])
                else:
                    nc.scalar.activation(h1g, h1_ps, AF.Gelu)
                return h1g

            def emit_mlp_w2(h1g, mp):
                # partial h2 for ALL examples using this core's I-shard,
                # then an 8-core ReduceScatter hands each core its own
                # example's 16 rows
                h2_ps = mp.tile([128, H], F32, tag="h2")
                for j in range(JS):
                    for off, ln in ((0, 512), (512, 256)):
                        nc.tensor.matmul(
                            h2_ps[:, off:off + ln], h1g[:, j],
                            w2_sb[:, j, off:off + ln],
                            start=(j == 0), stop=(j == JS - 1))
                h2sb = wpool.tile([128, H], BF, tag="h2sb")
                nc.scalar.copy(h2sb, h2_ps)
                nc.sync.dma_start(h2p_d[:], h2sb)
                nc.gpsimd.collective_compute(
                    "ReduceScatter", OP.add,
                    [[0, 1, 2, 3, 4, 5, 6, 7]],
                    ins=[h2p_d[:]], outs=[h2s_d[:]], cc_dim="Partition")
                z0 = wpool.tile([E, H], BF, tag="z0")
                nc.sync.dma_start(z0, h2s_d[:])
                # residual + LN over free dim (16 partitions)
                z = wpool.tile([E, H], F32, tag="z")
                nc.vector.tensor_add(z, z0, ext_t[:E])
                if use_bias.get("b2"):
                    nc.vector.tensor_add(z, z, bias_t["b2"][:E])
                stats = wpool.tile([E, 3, 6], F32, tag="st")
                for g in range(3):
                    nc.vector.bn_stats(stats[:, g],
                                       z[:, g * 256:(g + 1) * 256])
                mv = wpool.tile([E, 2], F32, tag="mv")
                nc.vector.bn_aggr(mv, stats)
                lnv = wpool.tile([E, 1], F32, tag="lnv")
                nc.scalar.activation(lnv, mv[:, 1:2], AF.Ln, bias=eps_t[:E])
                rs = wpool.tile([E, 1], F32, tag="rs")
                nc.scalar.activation(rs, lnv, AF.Exp, scale=-0.5)
                nc.vector.tensor_scalar(extLN[:E], z, mv[:, 0:1], rs,
                                        op0=OP.subtract, op1=OP.mult)
                if use_bias.get("mlp_g"):
                    nc.vector.tensor_mul(extLN[:E], extLN[:E],
                                         bias_t["mlp_g"][:E])
                if use_bias.get("mlp_b"):
                    nc.vector.tensor_add(extLN[:E], extLN[:E],
                                         bias_t["mlp_b"][:E])

            def emit_p2(mp):
                for k in range(KO):
                    tpPad(mp, extLNT[:, k], extLN[:, k * P:(k + 1) * P],
                          E, "act" if k % 2 else "dve")
                # a_t[:, k][h, e] = sum_h' W*[kh, h'] extLN[e, h']
                at_ps = mp.tile([128, KO, E], F32, tag="at")
                for k in range(KO):
                    for kp in range(KO):
                        nc.tensor.matmul(
                            at_ps[:, k],
                            wstarT_sb[:, kp, k * P:(k + 1) * P],
                            extLNT[:, kp], start=(kp == 0),
                            stop=(kp == KO - 1))
                nc.vector.tensor_copy(a_t, at_ps)
                # cvec[e] = bq . k_ext[e]  (general-bias path)
                if use_bias.get("wkbq"):
                    scrq = wpool.tile([E, H], F32, tag="cscr")
                    cv = wpool.tile([E, 1], F32, tag="cv")
                    nc.vector.tensor_mul(scrq, extLN[:E], bias_t["wkbq"][:E])
                    nc.vector.reduce_sum(cv, scrq, axis=mybir.AxisListType.X)
                    nc.vector.tensor_scalar_add(cv, cv, bias_t["bqbk"][:E])
                    cvp = mp.tile([128, 128], F32, tag="cvp")
                    nc.tensor.transpose(cvp[:1, :E], cv, ident_f[:E, :E])
                    cvr = wpool.tile([1, E], F32, tag="cvr")
                    nc.vector.tensor_copy(cvr, cvp[:1, :E])
                    nc.gpsimd.dma_start(cvec_bc, cvr.to_broadcast((128, E)))

            def emit_wv(mp):
                # wv' = gamma * (extLN @ Wvo) reusing the Wvo half of wcat
                wv_ps = mp.tile([E, 384], F32, tag="wv")
                for hf in range(2):
                    for k in range(KO):
                        nc.tensor.matmul(
                            wv_ps, extLNT[:, k],
                            wcat_sb[:, k, H + hf * 384:H + (hf + 1) * 384],
                            start=(k == 0), stop=(k == KO - 1))
                    if use_bias.get("bvwo"):
                        nc.vector.tensor_add(
                            wv_ps, wv_ps,
                            bias_t["bvwo"][:E, hf * 384:(hf + 1) * 384])
                    nc.vector.tensor_scalar_mul(
                        wvext[:E, hf * 384:(hf + 1) * 384], wv_ps, dl_t)

            def emit_se_all(mp):
                se_ps = mp.tile([128, E], F32, tag="se")
                for tt in range(TT):
                    for k in range(KO):
                        nc.tensor.matmul(se_ps,
                                         hsT[:, k, tt * P:(tt + 1) * P],
                                         a_t[:, k], start=(k == 0),
                                         stop=(k == KO - 1))
                    if use_bias.get("wkbq"):
                        nc.vector.tensor_add(se_sb[:, tt], se_ps, cvec_bc)
                    else:
                        nc.vector.tensor_copy(se_sb[:, tt], se_ps)

            def emit_Bs(tt):
                """softmax scalars for one token tile -> pg/p0 rings."""
                eext = work.tile([128, E], F32, tag="eext")
                zext = work.tile([128, 1], F32, tag="zext")
                nc.scalar.activation(eext, se_sb[:, tt], AF.Exp,
                                     accum_out=zext)
                e0 = work.tile([128, 1], F32, tag="e0")
                if use_bias.get("c0"):
                    nc.scalar.activation(e0, ss_all[:, tt:tt + 1], AF.Exp,
                                         bias=bias_t["c0"])
                else:
                    nc.scalar.activation(e0, ss_all[:, tt:tt + 1], AF.Exp)
                z_t = work.tile([128, 1], F32, tag="z")
                nc.vector.tensor_add(z_t, zext, e0)
                rz = work.tile([128, 1], F32, tag="rz")
                nc.vector.reciprocal(rz, z_t)
                nc.vector.tensor_mul(p0_ring[:, tt % 4:tt % 4 + 1], e0, rz)
                nc.vector.tensor_scalar_mul(pg_ring[:, tt % 4], eext, rz)

            def emit_Bt(tt, pgt_pool):
                """P@wv' + output dense tail for one token tile."""
                pgT_ps = pgt_pool.tile([E, 128], F32, tag="pgT")
                nc.tensor.matmul(pgT_ps, pg_ring[:, tt % 4], ident,
                                 start=True, stop=True)
                nc.scalar.copy(pgT_pad[:E], pgT_ps)
                # sb1 = p0 * ov   (ACT Copy+scale, from SBUF)
                sb1 = work.tile([128, H], BF, tag="sb1")
                nc.scalar.activation(sb1, ov_sb[:, tt], AF.Copy,
                                     scale=p0_ring[:, tt % 4:tt % 4 + 1])
                # Pv overwrites banks 1-2 of a retired uo buffer: address
                # overlap with both the u tail and the ov head orders it
                # after every reader of that buffer
                pv = uo_pool.tile([128, H2], F32, tag="uo")
                for off, ln in ((512, 512), (1024, 256)):
                    nc.tensor.matmul(pv[:, off:off + ln], pgT_pad,
                                     wvext[:, off - 512:off - 512 + ln],
                                     start=True, stop=True)
                out2 = work.tile([128, H], BF, tag="out2")
                nc.vector.tensor_add(out2, sb1, pv[:, 512:512 + H])
                if use_bias.get("bo"):
                    nc.vector.tensor_add(out2, out2, bias_t["bo"])
                sbz = work.tile([128, H], BF, tag="sbz")
                nc.gpsimd.tensor_add(sbz, out2, hs_sb[:, tt])
                # LayerNorm over H; rstd = Exp(-0.5 * Ln(var + eps))
                stats = work.tile([128, 2, 6], F32, tag="lnst")
                for g in range(2):
                    nc.vector.bn_stats(stats[:, g],
                                       sbz[:, g * 384:(g + 1) * 384])
                mv = work.tile([128, 2], F32, tag="lnmv")
                nc.vector.bn_aggr(mv, stats)
                lnv = work.tile([128, 1], F32, tag="lnv")
                nc.scalar.activation(lnv, mv[:, 1:2], AF.Ln, bias=eps_t)
                rs = work.tile([128, 1], F32, tag="lnrs")
                nc.scalar.activation(rs, lnv, AF.Exp, scale=-0.5)
                fin = work.tile([128, H], BF, tag="fin")
                nc.vector.tensor_scalar(fin, sbz, mv[:, 0:1], rs,
                                        op0=OP.subtract, op1=OP.mult)
                if use_bias.get("ln_g"):
                    nc.vector.tensor_mul(fin, fin, bias_t["ln_g"])
                if use_bias.get("ln_b"):
                    nc.vector.tensor_add(fin, fin, bias_t["ln_b"])
                nc.scalar.dma_start(out_r[:, tt], fin)

            # ---------------- schedule ----------------
            stg = {"dma": 0, "A": 1, "mlp": 2, "se": 3, "full": 4}[_STAGE]
            with tc.tile_pool(name="tpX", bufs=1, space="PSUM") as tpx:
                for k in range(KO):
                    tp128(tpx, extT[:, k], ext_all_sb[:, k * P:(k + 1) * P],
                          "act" if k % 2 else "dve")
            if stg >= 2:
                with tc.tile_pool(name="mlp1", bufs=1, space="PSUM") as mp:
                    h1g = emit_mlp_w1(mp)
                with tc.tile_pool(name="mlp2", bufs=1, space="PSUM") as mp:
                    emit_mlp_w2(h1g, mp)
            with tc.tile_pool(name="tpA", bufs=2, space="PSUM") as tpa:
                emit_hsT(0, tpa)
                emit_hsT(1, tpa)
                if stg >= 1:
                    emit_A(0)
                emit_hsT(2, tpa)
                emit_hsT(3, tpa)
                if stg >= 1:
                    emit_A(1)
                emit_hsT(4, tpa)
                emit_hsT(5, tpa)
                if stg >= 1:
                    emit_A(2)
                emit_hsT(6, tpa)
                emit_hsT(7, tpa)
                if stg >= 1:
                    emit_A(3)
                    emit_A(4)
            if stg >= 1:
                emit_A(5)
                emit_A(6)
            with tc.tile_pool(name="tpB", bufs=2, space="PSUM") as tpb:
                for tt in range(8, TT):
                    emit_hsT(tt, tpb)
            if stg >= 1:
                emit_A(7)
                emit_A(8)
                emit_A(9)
            if stg >= 2:
                with tc.tile_pool(name="p2a", bufs=1, space="PSUM") as mp:
                    emit_p2(mp)
                with tc.tile_pool(name="p2b", bufs=1, space="PSUM") as mp:
                    emit_wv(mp)
            if stg >= 3:
                with tc.tile_pool(name="sep", bufs=1, space="PSUM") as mp:
                    emit_se_all(mp)
            if stg >= 4:
                with tc.tile_pool(name="pgt_ps", bufs=1,
                                  space="PSUM") as pgt_pool:
                    sq = list(range(TT))   # softmax pending
                    tq = []                # tail pending
                    for tt in range(10, TT):
                        emit_A(tt)
                        for _ in range(2):
                            if tq:
                                emit_Bt(tq.pop(0), pgt_pool)
                        for _ in range(2):
                            if sq and sq[0] <= tt:
                                j = sq.pop(0)
                                emit_Bs(j)
                                tq.append(j)
                    while sq or tq:
                        if tq:
                            emit_Bt(tq.pop(0), pgt_pool)
                        if sq:
                            j = sq.pop(0)
                            emit_Bs(j)
                            tq.append(j)
            else:
                if stg >= 1:
                    for tt in range(10, TT):
                        emit_A(tt)
                for tt in range(TT):
                    src = ov_sb[:, tt] if stg >= 1 else hs_sb[:, tt]
                    nc.scalar.dma_start(out_r[:, tt], src)
